# revision 7
# baseline (speedup 1.0000x reference)
"""Multi-head attention kernel for Trainium2 (8 NeuronCores).

Problem: B=4, T=2048, U=1024, H=16 heads, D=64. Full (non-causal) softmax
attention per head. 64 independent (head, batch) problems.

Sharding: core c owns batch b = c//2 and head block hb = c%2 (8 contiguous
heads = 512 contiguous channels). No cross-core communication.

v2 design (host-layout-assisted, ACT+DVE split softmax):
  - The HOST pre-transposes each core's Q and K shard to [DD=512, T=2048]
    (d-major) so the kernel loads QT/KT directly -- no on-device transposes
    at all. V stays [T, DD]. The output is produced d-major [DD, T] and the
    host transposes it back during unshard.
  - Preamble per core: DMA 12 MiB fp32 (QT, KT, V), convert to bf16 on
    GpSimd. V is converted into per-t-chunk [128, 8*65] tiles where each
    head's 64 columns get a ones column appended (computes the softmax
    denominator for free in mm2).
  - Main loop processes a HEAD PAIR at a time; the two mm1 matmuls (K=64
    contraction) run CONCURRENTLY as row-tiled matmuls at tile_position
    (0,0)/(64,0), writing sc[128 k, 1024 = 2*512 q] in alternating PSUM
    bank-groups.
  - exp(sc/8) is split across TWO engines: most k-chunks use the ACT
    engine (true exp -> bf16, ~1.15us per [128,1024]); a tunable subset
    (DVE_N of 16) is computed on the Vector engine with a one-instruction
    Schraudolph approximation: i16 = int16(s*23.083 + 16250) IS the bf16
    bit pattern of exp(s/8)*(1+eps), |eps|<~3%. The eps sawtooth is
    zero-mean and cancels in the softmax ratio to ~1% output rel-err at
    DVE_N=8 (measured 1.1e-2 end to end vs 2e-2 budget).
  - mm2 (sw-pipelined `depth` chunks behind mm1/exp):
      outpA[65, 512] += V_A_aug[kc].T @ pb[:, 0:512]
      outpB[65, 512] += V_B_aug[kc].T @ pb[:, 512:1024]
  - Epilogue per head: r = 1/outp[64] (DVE, from PSUM); partition-broadcast
    (GpSimd); out = outp[0:64] * r (DVE, PSUM x SBUF); fp32 store d-major
    with 2 KiB contiguous lines. No hi/lo split, no DMA transposes.
"""

import os
import sys

sys.path.insert(0, "/opt/trn_rl_repo")

import numpy as np

import concourse.bass as bass
import concourse.bacc as bacc
import concourse.mybir as mybir
import concourse.tile as tile
from concourse import library_config
from concourse.bass_utils import run_bass_kernel_spmd

F32 = mybir.dt.float32
BF16 = mybir.dt.bfloat16
I16 = mybir.dt.int16
EXP = mybir.ActivationFunctionType.Exp
MULT = mybir.AluOpType.mult
ADD = mybir.AluOpType.add

B, T, U = 4, 2048, 1024
H_TOTAL, D = 16, 64
DD = 512          # channels per core (8 heads)
H = 8             # heads per core
KC = 16           # k chunks of 128
TC = 16           # t chunks of 128
HP = 4            # head pairs
NQQ = 4           # q quarters
QQ = 512          # q per quarter
N_CORES = 8
SCALE = 1.0 / 8.0  # 1/sqrt(D)

# Schraudolph constants: bf16 bits of exp(s*SCALE) ~= s*A + B (as int16)
SCH_A = SCALE * 128.0 / float(np.log(2.0))   # 23.0831...
SCH_B = 16256.0 - 5.8

DEPTH_DEFAULT = int(os.environ.get("K_DEPTH", "3"))
DVE_N_DEFAULT = int(os.environ.get("DVE_N", "6"))


def _dve_kcs(n):
    if n <= 0:
        return set()
    if n <= KC // 2:
        # odd kcs first: perfect ACT/DVE alternation overlaps the two
        # engines within the sc-tile recycling chain
        return set(range(1, 2 * n, 2))
    return set(range(1, KC, 2)) | set(range(0, 2 * (n - KC // 2), 2))


def build_program(nc, bench_iters=0, stages=("mm1", "exp", "mm2", "norm"),
                  depth=None, dve_n=None):
    if depth is None:
        depth = DEPTH_DEFAULT
    if dve_n is None:
        dve_n = DVE_N_DEFAULT
    dve_kcs = _dve_kcs(dve_n)
    if bench_iters:
        # Timing-only variant: big tensors are Internal (values irrelevant),
        # external I/O is tiny, and the whole body runs in a For_i loop.
        in_flag = nc.dram_tensor("in_flag", [1, 1], F32, kind="ExternalInput").ap()
        out_flag = nc.dram_tensor("out_flag", [1, 1], F32, kind="ExternalOutput").ap()
        qT_d = nc.dram_tensor("querysT", [DD, T], F32).ap()
        kT_d = nc.dram_tensor("keysT", [DD, T], F32).ap()
        v_d = nc.dram_tensor("values", [T, DD], F32).ap()
        o_d = nc.dram_tensor("out", [DD, T], F32).ap()
    else:
        qT_d = nc.dram_tensor("querysT", [DD, T], F32, kind="ExternalInput").ap()
        kT_d = nc.dram_tensor("keysT", [DD, T], F32, kind="ExternalInput").ap()
        v_d = nc.dram_tensor("values", [T, DD], F32, kind="ExternalInput").ap()
        o_d = nc.dram_tensor("out", [DD, T], F32, kind="ExternalOutput").ap()

    import contextlib

    with tile.TileContext(nc) as tc:
        if bench_iters:
            nc.sync.dma_start(o_d[0:1, 0:1], in_flag[:])  # consume input
            loop_cm = tc.For_i(0, bench_iters, 1)
        else:
            loop_cm = contextlib.nullcontext()
        with (
            tc.tile_pool(name="persist", bufs=1) as persist,
            tc.tile_pool(name="stage", bufs=3) as stage,
            tc.tile_pool(name="probs", bufs=max(3, depth + 2)) as probs_pool,
            tc.tile_pool(name="norm", bufs=2) as norm_pool,
            tc.tile_pool(name="ps_sce", bufs=1, space=bass.MemorySpace.PSUM) as ps_sce,
            tc.tile_pool(name="ps", bufs=2, space=bass.MemorySpace.PSUM) as ps,
            tc.tile_pool(name="ps_sco", bufs=1, space=bass.MemorySpace.PSUM) as ps_sco,
        ):
            with loop_cm:
                nc.gpsimd.load_library(library_config.attn)

                # Dummy exp to hoist the ACT table load to t=0.
                warm = persist.tile([1, 1], F32, tag="warm")
                nc.gpsimd.memset(warm[:], 0.0)
                warm_o = persist.tile([1, 1], F32, tag="warm_o")
                nc.scalar.activation(warm_o[:], warm[:], EXP)

                # persistent tiles
                vc = [
                    persist.tile([128, H * 65], BF16, tag=f"vc{c}", name=f"vc{c}")
                    for c in range(TC)
                ]
                for c in range(TC):
                    nc.gpsimd.memset(
                        vc[c][:].rearrange("p (h e) -> p h e", e=65)[:, :, 64:65], 1.0
                    )
                qt = [
                    persist.tile([128, T], BF16, tag=f"qt{hp}", name=f"qt{hp}")
                    for hp in range(HP)
                ]
                kt = [
                    persist.tile([128, T], BF16, tag=f"kt{hp}", name=f"kt{hp}")
                    for hp in range(HP)
                ]
                v_3d = v_d.rearrange("(c p) d -> c p d", p=128)
                q3 = qT_d.rearrange("(hp p) t -> hp p t", p=128)
                k3 = kT_d.rearrange("(hp p) t -> hp p t", p=128)

                def qk_chunk(src3, hp, which):
                    s = stage.tile([128, T], F32, tag="qkstage",
                                   name=f"{which}s{hp}")
                    nc.sync.dma_start(s[:], src3[hp])
                    dst = (kt if which == "k" else qt)[hp]
                    nc.gpsimd.tensor_copy(dst[:], s[:])

                def v_chunk(c):
                    vs = stage.tile([128, DD], F32, tag="vstage", bufs=4,
                                    name=f"vs{c}")
                    nc.sync.dma_start(vs[:], v_3d[c])
                    nc.gpsimd.tensor_copy(
                        vc[c][:].rearrange("p (h e) -> p h e", e=65)[:, :, 0:64],
                        vs[:].rearrange("p (h e) -> p h e", e=64),
                    )

                # preamble: hp0's K/Q first so mm1 can start ASAP
                qk_chunk(k3, 0, "k")
                qk_chunk(q3, 0, "q")
                for c in range(4):
                    v_chunk(c)
                for hp in range(1, HP):
                    qk_chunk(k3, hp, "k")
                    qk_chunk(q3, hp, "q")
                    for c in range(4 * hp, 4 * hp + 4):
                        v_chunk(c)

                # ---- main loop: one head PAIR at a time ----
                # norm work for block n is EMITTED during block n+1 (after a
                # few chunks) so its DVE/queue waits are already satisfied
                # when it reaches each engine's strict-FIFO queue head.
                pending_norm = []

                def flush_norm():
                    while pending_norm:
                        pending_norm.pop(0)()

                for hp in range(HP):
                    hA, hB = 2 * hp, 2 * hp + 1
                    for qq in range(NQQ):
                        qsl = slice(qq * QQ, (qq + 1) * QQ)
                        outpA = ps.tile([65, QQ], F32, tag="outpA", name="outpA")
                        outpB = ps.tile([65, QQ], F32, tag="outpB", name="outpB")
                        pbq = {}
                        for kci in range(KC + depth):
                            if kci == 4:
                                flush_norm()
                            if kci < KC and "mm1" in stages:
                                kc = kci
                                # alternate PSUM bank groups so exp reads
                                # never share a group with the next chunk's
                                # mm1 writes
                                pool = ps_sce if kc % 2 == 0 else ps_sco
                                sc = pool.tile([128, 1024], F32, tag="sc",
                                               name="sc")
                                ksl = slice(kc * 128, (kc + 1) * 128)
                                nc.tensor.matmul(
                                    sc[:, 0:512],
                                    kt[hp][0:64, ksl],
                                    qt[hp][0:64, qsl],
                                    start=True,
                                    stop=True,
                                    tile_position=(0, 0),
                                )
                                nc.tensor.matmul(
                                    sc[:, 512:1024],
                                    kt[hp][64:128, ksl],
                                    qt[hp][64:128, qsl],
                                    start=True,
                                    stop=True,
                                    tile_position=(64, 0),
                                )
                                if "exp" in stages:
                                    if kc in dve_kcs:
                                        pb = probs_pool.tile(
                                            [128, 1024], I16, tag="pb16",
                                            name="pb16"
                                        )
                                        nc.vector.tensor_scalar(
                                            pb[:], sc[:], SCH_A, SCH_B,
                                            MULT, ADD,
                                        )
                                        pbq[kc] = (pb, True)
                                    else:
                                        pb = probs_pool.tile(
                                            [128, 1024], BF16, tag="pb",
                                            name="pb"
                                        )
                                        nc.scalar.activation(
                                            pb[:], sc[:], EXP, scale=SCALE
                                        )
                                        pbq[kc] = (pb, False)
                            kc2 = kci - depth
                            if "mm2" in stages and 0 <= kc2 < KC:
                                pb2, is16 = pbq.pop(kc2)
                                rA = pb2[:, 0:512]
                                rB = pb2[:, 512:1024]
                                if is16:
                                    rA = rA.bitcast(BF16)
                                    rB = rB.bitcast(BF16)
                                nc.tensor.matmul(
                                    outpA[:],
                                    vc[kc2][:, hA * 65 : (hA + 1) * 65],
                                    rA,
                                    start=(kc2 == 0),
                                    stop=(kc2 == KC - 1),
                                )
                                nc.tensor.matmul(
                                    outpB[:],
                                    vc[kc2][:, hB * 65 : (hB + 1) * 65],
                                    rB,
                                    start=(kc2 == 0),
                                    stop=(kc2 == KC - 1),
                                )
                        if "mm2" not in stages or "norm" not in stages:
                            continue

                        def make_norm(h, outp, qsl):
                            def emit():
                                r = norm_pool.tile([1, QQ], F32, tag="r",
                                                   name="r")
                                nc.vector.reciprocal(r[:], outp[64:65, :])
                                bc = norm_pool.tile([64, QQ], F32, tag="bc",
                                                    name="bc")
                                nc.gpsimd.partition_broadcast(bc[:], r[:])
                                ob = norm_pool.tile([64, QQ], F32, tag="ob",
                                                    name="ob")
                                nc.vector.tensor_mul(ob[:], outp[0:64, :], bc[:])
                                # store via SWDGE (Pool): separate DMA queue,
                                # so the SP HWDGE queue stays free for the
                                # next iteration's loads
                                nc.gpsimd.dma_start(
                                    o_d[h * 64 : (h + 1) * 64, qsl], ob[:]
                                )
                            return emit

                        pending_norm.append(make_norm(hA, outpA, qsl))
                        pending_norm.append(make_norm(hB, outpB, qsl))
                flush_norm()
        if bench_iters:
            nc.sync.dma_start(out_flag[:], o_d[0:1, 0:1])
    return nc


_CACHED = None


def _get_program():
    global _CACHED
    if _CACHED is None:
        nc = bacc.Bacc("TRN2", target_bir_lowering=False, debug=False)
        _CACHED = build_program(nc)
        _CACHED.compile()
    return _CACHED


def _make_in_maps(querys, keys, values):
    querys = np.asarray(querys, dtype=np.float32)
    keys = np.asarray(keys, dtype=np.float32)
    values = np.asarray(values, dtype=np.float32)
    in_maps = []
    for c in range(N_CORES):
        b, hb = c // 2, c % 2
        sl = slice(hb * DD, (hb + 1) * DD)
        in_maps.append(
            {
                "querysT": np.ascontiguousarray(querys[b, :, sl].T),
                "keysT": np.ascontiguousarray(keys[b, :, sl].T),
                "values": np.ascontiguousarray(values[b, :, sl]),
            }
        )
    return in_maps


def kernel(querys, keys, values):
    nc = _get_program()
    in_maps = _make_in_maps(querys, keys, values)
    res = run_bass_kernel_spmd(nc, in_maps, list(range(N_CORES)))
    out = np.empty((B, T, U), dtype=np.float32)
    for c in range(N_CORES):
        b, hb = c // 2, c % 2
        out[b, :, hb * DD : (hb + 1) * DD] = res.results[c]["out"].T
    return out


# revision 11
# speedup vs baseline: 1.1158x; 1.1158x over previous
"""Multi-head attention kernel for Trainium2 (8 NeuronCores).

Problem: B=4, T=2048, U=1024, H=16 heads, D=64. Full (non-causal) softmax
attention per head. 64 independent (head, batch) problems.

Sharding: core c owns batch b = c//2 and head block hb = c%2 (8 contiguous
heads = 512 contiguous channels). No cross-core communication.

v2 design (host-layout-assisted, ACT+DVE split softmax):
  - The HOST pre-transposes each core's Q and K shard to [DD=512, T=2048]
    (d-major) so the kernel loads QT/KT directly -- no on-device transposes
    at all. V stays [T, DD]. The output is produced d-major [DD, T] and the
    host transposes it back during unshard.
  - Preamble per core: DMA 12 MiB fp32 (QT, KT, V), convert to bf16 on
    GpSimd. V is converted into per-t-chunk [128, 8*65] tiles where each
    head's 64 columns get a ones column appended (computes the softmax
    denominator for free in mm2).
  - Main loop processes a HEAD PAIR at a time; the two mm1 matmuls (K=64
    contraction) run CONCURRENTLY as row-tiled matmuls at tile_position
    (0,0)/(64,0), writing sc[128 k, 1024 = 2*512 q] in alternating PSUM
    bank-groups.
  - exp(sc/8) is split across TWO engines: most k-chunks use the ACT
    engine (true exp -> bf16, ~1.15us per [128,1024]); a tunable subset
    (DVE_N of 16) is computed on the Vector engine with a one-instruction
    Schraudolph approximation: i16 = int16(s*23.083 + 16250) IS the bf16
    bit pattern of exp(s/8)*(1+eps), |eps|<~3%. The eps sawtooth is
    zero-mean and cancels in the softmax ratio to ~1% output rel-err at
    DVE_N=8 (measured 1.1e-2 end to end vs 2e-2 budget).
  - mm2 (sw-pipelined `depth` chunks behind mm1/exp):
      outpA[65, 512] += V_A_aug[kc].T @ pb[:, 0:512]
      outpB[65, 512] += V_B_aug[kc].T @ pb[:, 512:1024]
  - Epilogue per head: r = 1/outp[64] (DVE, from PSUM); partition-broadcast
    (GpSimd); out = outp[0:64] * r (DVE, PSUM x SBUF); fp32 store d-major
    with 2 KiB contiguous lines. No hi/lo split, no DMA transposes.
"""

import os
import sys

sys.path.insert(0, "/opt/trn_rl_repo")

import numpy as np

import concourse.bass as bass
import concourse.bacc as bacc
import concourse.mybir as mybir
import concourse.tile as tile
from concourse import library_config
from concourse.bass_utils import run_bass_kernel_spmd

F32 = mybir.dt.float32
BF16 = mybir.dt.bfloat16
I16 = mybir.dt.int16
EXP = mybir.ActivationFunctionType.Exp
MULT = mybir.AluOpType.mult
ADD = mybir.AluOpType.add
DIVIDE = mybir.AluOpType.divide

B, T, U = 4, 2048, 1024
H_TOTAL, D = 16, 64
DD = 512          # channels per core (8 heads)
H = 8             # heads per core
KC = 16           # k chunks of 128
TC = 16           # t chunks of 128
HP = 4            # head pairs
NQQ = 4           # q quarters
QQ = 512          # q per quarter
N_CORES = 8
SCALE = 1.0 / 8.0  # 1/sqrt(D)

# Schraudolph constants: bf16 bits of exp(s*SCALE) ~= s*A + B (as int16)
SCH_A = SCALE * 128.0 / float(np.log(2.0))   # 23.0831...
SCH_B = 16256.0 - 5.8

DEPTH_DEFAULT = int(os.environ.get("K_DEPTH", "3"))
DVE_N_DEFAULT = int(os.environ.get("DVE_N", "6"))


def _dve_kcs(n):
    if n <= 0:
        return set()
    if n <= KC // 2:
        # odd kcs first: perfect ACT/DVE alternation overlaps the two
        # engines within the sc-tile recycling chain
        return set(range(1, 2 * n, 2))
    return set(range(1, KC, 2)) | set(range(0, 2 * (n - KC // 2), 2))


def build_program(nc, bench_iters=0, stages=("mm1", "exp", "mm2", "norm"),
                  depth=None, dve_n=None):
    if depth is None:
        depth = DEPTH_DEFAULT
    if dve_n is None:
        dve_n = DVE_N_DEFAULT
    dve_kcs = _dve_kcs(dve_n)
    if bench_iters:
        # Timing-only variant: big tensors are Internal (values irrelevant),
        # external I/O is tiny, and the whole body runs in a For_i loop.
        in_flag = nc.dram_tensor("in_flag", [1, 1], F32, kind="ExternalInput").ap()
        out_flag = nc.dram_tensor("out_flag", [1, 1], F32, kind="ExternalOutput").ap()
        qT_d = nc.dram_tensor("querysT", [DD, T], F32).ap()
        kT_d = nc.dram_tensor("keysT", [DD, T], F32).ap()
        v_d = nc.dram_tensor("values", [T, DD], F32).ap()
        o_d = nc.dram_tensor("out", [DD, T], F32).ap()
    else:
        qT_d = nc.dram_tensor("querysT", [DD, T], F32, kind="ExternalInput").ap()
        kT_d = nc.dram_tensor("keysT", [DD, T], F32, kind="ExternalInput").ap()
        v_d = nc.dram_tensor("values", [T, DD], F32, kind="ExternalInput").ap()
        o_d = nc.dram_tensor("out", [DD, T], F32, kind="ExternalOutput").ap()

    import contextlib

    with tile.TileContext(nc) as tc:
        if bench_iters:
            nc.sync.dma_start(o_d[0:1, 0:1], in_flag[:])  # consume input
            loop_cm = tc.For_i(0, bench_iters, 1)
        else:
            loop_cm = contextlib.nullcontext()
        with (
            tc.tile_pool(name="persist", bufs=1) as persist,
            tc.tile_pool(name="stage", bufs=3) as stage,
            tc.tile_pool(name="probs", bufs=max(3, depth + 2)) as probs_pool,
            tc.tile_pool(name="norm", bufs=2) as norm_pool,
            tc.tile_pool(name="ps_sce", bufs=1, space=bass.MemorySpace.PSUM) as ps_sce,
            tc.tile_pool(name="ps", bufs=2, space=bass.MemorySpace.PSUM) as ps,
            tc.tile_pool(name="ps_sco", bufs=1, space=bass.MemorySpace.PSUM) as ps_sco,
        ):
            with loop_cm:
                nc.gpsimd.load_library(library_config.attn)

                # Dummy exp to hoist the ACT table load to t=0.
                warm = persist.tile([1, 1], F32, tag="warm")
                nc.gpsimd.memset(warm[:], 0.0)
                warm_o = persist.tile([1, 1], F32, tag="warm_o")
                nc.scalar.activation(warm_o[:], warm[:], EXP)

                # persistent tiles
                vc = [
                    persist.tile([128, H * 65], BF16, tag=f"vc{c}", name=f"vc{c}")
                    for c in range(TC)
                ]
                for c in range(TC):
                    nc.gpsimd.memset(
                        vc[c][:].rearrange("p (h e) -> p h e", e=65)[:, :, 64:65], 1.0
                    )
                qt = [
                    persist.tile([128, T], BF16, tag=f"qt{hp}", name=f"qt{hp}")
                    for hp in range(HP)
                ]
                kt = [
                    persist.tile([128, T], BF16, tag=f"kt{hp}", name=f"kt{hp}")
                    for hp in range(HP)
                ]
                v_3d = v_d.rearrange("(c p) d -> c p d", p=128)
                q3 = qT_d.rearrange("(hp p) t -> hp p t", p=128)
                k3 = kT_d.rearrange("(hp p) t -> hp p t", p=128)

                def qk_chunk(src3, hp, which):
                    s = stage.tile([128, T], F32, tag="qkstage",
                                   name=f"{which}s{hp}")
                    nc.sync.dma_start(s[:], src3[hp])
                    dst = (kt if which == "k" else qt)[hp]
                    nc.gpsimd.tensor_copy(dst[:], s[:])

                def v_chunk(c):
                    vs = stage.tile([128, DD], F32, tag="vstage", bufs=4,
                                    name=f"vs{c}")
                    nc.sync.dma_start(vs[:], v_3d[c])
                    nc.gpsimd.tensor_copy(
                        vc[c][:].rearrange("p (h e) -> p h e", e=65)[:, :, 0:64],
                        vs[:].rearrange("p (h e) -> p h e", e=64),
                    )

                # preamble: hp0's K/Q first so mm1 can start ASAP
                qk_chunk(k3, 0, "k")
                qk_chunk(q3, 0, "q")
                for c in range(4):
                    v_chunk(c)
                for hp in range(1, HP):
                    qk_chunk(k3, hp, "k")
                    qk_chunk(q3, hp, "q")
                    for c in range(4 * hp, 4 * hp + 4):
                        v_chunk(c)

                # ---- main loop: one head PAIR at a time ----
                # norm work for block n is EMITTED during block n+1 (after a
                # few chunks) so its DVE/queue waits are already satisfied
                # when it reaches each engine's strict-FIFO queue head.
                pending_norm = []

                def flush_norm():
                    while pending_norm:
                        pending_norm.pop(0)()

                for hp in range(HP):
                    hA, hB = 2 * hp, 2 * hp + 1
                    for qq in range(NQQ):
                        qsl = slice(qq * QQ, (qq + 1) * QQ)
                        outpA = ps.tile([65, QQ], F32, tag="outpA", name="outpA")
                        outpB = ps.tile([65, QQ], F32, tag="outpB", name="outpB")
                        pbq = {}
                        for kci in range(KC + depth):
                            if kci == 4:
                                flush_norm()
                            if kci < KC and "mm1" in stages:
                                kc = kci
                                # alternate PSUM bank groups so exp reads
                                # never share a group with the next chunk's
                                # mm1 writes
                                pool = ps_sce if kc % 2 == 0 else ps_sco
                                sc = pool.tile([128, 1024], F32, tag="sc",
                                               name="sc")
                                ksl = slice(kc * 128, (kc + 1) * 128)
                                nc.tensor.matmul(
                                    sc[:, 0:512],
                                    kt[hp][0:64, ksl],
                                    qt[hp][0:64, qsl],
                                    start=True,
                                    stop=True,
                                    tile_position=(0, 0),
                                )
                                nc.tensor.matmul(
                                    sc[:, 512:1024],
                                    kt[hp][64:128, ksl],
                                    qt[hp][64:128, qsl],
                                    start=True,
                                    stop=True,
                                    tile_position=(64, 0),
                                )
                                if "exp" in stages:
                                    if kc in dve_kcs:
                                        pb = probs_pool.tile(
                                            [128, 1024], BF16, tag="pb16",
                                            name="pb16"
                                        )
                                        nc.vector.tensor_scalar(
                                            pb[:].bitcast(I16), sc[:],
                                            SCH_A, SCH_B, MULT, ADD,
                                        )
                                        pbq[kc] = (pb, False)
                                    else:
                                        pb = probs_pool.tile(
                                            [128, 1024], BF16, tag="pb",
                                            name="pb"
                                        )
                                        nc.scalar.activation(
                                            pb[:], sc[:], EXP, scale=SCALE
                                        )
                                        pbq[kc] = (pb, False)
                            kc2 = kci - depth
                            if "mm2" in stages and 0 <= kc2 < KC:
                                pb2, is16 = pbq.pop(kc2)
                                rA = pb2[:, 0:512]
                                rB = pb2[:, 512:1024]
                                if is16:
                                    rA = rA.bitcast(BF16)
                                    rB = rB.bitcast(BF16)
                                nc.tensor.matmul(
                                    outpA[:],
                                    vc[kc2][:, hA * 65 : (hA + 1) * 65],
                                    rA,
                                    start=(kc2 == 0),
                                    stop=(kc2 == KC - 1),
                                )
                                nc.tensor.matmul(
                                    outpB[:],
                                    vc[kc2][:, hB * 65 : (hB + 1) * 65],
                                    rB,
                                    start=(kc2 == 0),
                                    stop=(kc2 == KC - 1),
                                )
                        if "mm2" not in stages or "norm" not in stages:
                            continue

                        def make_norm(h, outp, qsl):
                            def emit():
                                # DVE does only a tiny reciprocal (its PSUM
                                # wait is long-satisfied when it reaches the
                                # queue head); ACT evacuates the numerator;
                                # GpSimd broadcasts and multiplies.
                                r = norm_pool.tile([1, QQ], F32, tag="r",
                                                   name="r")
                                nc.vector.reciprocal(r[:], outp[64:65, :])
                                outsb = norm_pool.tile([64, QQ], F32,
                                                       tag="outsb",
                                                       name="outsb")
                                nc.scalar.copy(outsb[:], outp[0:64, :])
                                bc = norm_pool.tile([64, QQ], F32, tag="bc",
                                                    name="bc")
                                nc.gpsimd.partition_broadcast(bc[:], r[:])
                                ob = norm_pool.tile([64, QQ], F32, tag="ob",
                                                    name="ob")
                                nc.gpsimd.tensor_tensor(
                                    ob[:], outsb[:], bc[:], MULT
                                )
                                # store via SWDGE (Pool): separate DMA queue,
                                # so the SP HWDGE queue stays free for the
                                # next iteration's loads
                                nc.gpsimd.dma_start(
                                    o_d[h * 64 : (h + 1) * 64, qsl], ob[:]
                                )
                            return emit

                        pending_norm.append(make_norm(hA, outpA, qsl))
                        pending_norm.append(make_norm(hB, outpB, qsl))
                flush_norm()
        if bench_iters:
            nc.sync.dma_start(out_flag[:], o_d[0:1, 0:1])
    return nc


_CACHED = None


def _get_program():
    global _CACHED
    if _CACHED is None:
        nc = bacc.Bacc("TRN2", target_bir_lowering=False, debug=False)
        _CACHED = build_program(nc)
        _CACHED.compile()
    return _CACHED


def _make_in_maps(querys, keys, values):
    querys = np.asarray(querys, dtype=np.float32)
    keys = np.asarray(keys, dtype=np.float32)
    values = np.asarray(values, dtype=np.float32)
    in_maps = []
    for c in range(N_CORES):
        b, hb = c // 2, c % 2
        sl = slice(hb * DD, (hb + 1) * DD)
        in_maps.append(
            {
                "querysT": np.ascontiguousarray(querys[b, :, sl].T),
                "keysT": np.ascontiguousarray(keys[b, :, sl].T),
                "values": np.ascontiguousarray(values[b, :, sl]),
            }
        )
    return in_maps


def kernel(querys, keys, values):
    nc = _get_program()
    in_maps = _make_in_maps(querys, keys, values)
    res = run_bass_kernel_spmd(nc, in_maps, list(range(N_CORES)))
    out = np.empty((B, T, U), dtype=np.float32)
    for c in range(N_CORES):
        b, hb = c // 2, c % 2
        out[b, :, hb * DD : (hb + 1) * DD] = res.results[c]["out"].T
    return out
